# revision 22
# baseline (speedup 1.0000x reference)
"""Trainium2 Bass kernel for nn_Colar_static (retrieval_knn).

Data-parallel over batch B across 8 cores + tensor-parallel split of
the Ek/Ev prototype projections over C=1024 (each core a [128, K*N]
slab), slabs exchanged with on-chip collectives.

v2 changes vs the 304us baseline:
  - fp8 (e4m3) DoubleRow matmuls (0.5 cyc/row) for the Ek/Ev slab
    projections, the K projection, SIM and FE. Weights host-scaled by
    32 (un-scaled via activation `scale`); kn scaled by 16, wf by 128.
    V projection + out projection stay bf16: the v-half dominates the
    output (out std 0.425 of 0.432) so its precision gates accuracy,
    while everything downstream of the softmax is attenuated ~30x.
  - exactly 2 activation-table loads: sqrt_and_others for P/KV
    (Identity/Square/Relu/Sqrt), exp_and_others for SIM/FE/OUT
    (Exp/Tanh/Identity/Copy/Relu). sigmoid -> 0.5+0.5*tanh(x/2),
    1/sum -> DVE reciprocal. Table switch is warmed mid-KV.
  - Ek AllGather issued right after the Ek pass (all other bulk HWDGE
    pre-issued; stat/x/w all SBUF-resident in fp8), fp8 gather payloads.
  - gse/gtg merged into one [P,2]-stationary matmul; bcs broadcast
    deferred one k-iteration so PE never stalls on the gate chain;
    wf/kn multiplies read PSUM broadcasts directly on DVE.
"""

import sys

for _p in ("/opt/trn_rl_repo", "/opt/pypackages"):
    if _p not in sys.path:
        sys.path.append(_p)

import numpy as np
import ml_dtypes

import concourse.bass as bass
import concourse.mybir as mybir
import concourse.tile as tile
from concourse import bacc
from concourse import bass_utils

B, T, CH, C, N, K = 4096, 8, 2048, 1024, 512, 5
NCORES = 8
BL = B // NCORES            # 512 batch rows per core
KN = K * N                  # 2560 prototype columns
P = 128
NT_I = CH // P              # 16 contraction tiles (input channels)
NT_C = C // P               # 8 tiles over C
NT_KN = KN // P             # 20 tiles over K*N
NT_KV = 2 * C // P          # 16 tiles over [k|v] output channels
TPK = NT_KN // K            # 4 kn-tiles per prototype
NCH = KN // 512             # 5 column chunks for the slab projections
EPS = 1e-8
WS = 32.0                   # fp8 weight prescale
KNS = 16.0                  # kn prescale
WFS = 128.0                 # wf prescale

F32 = mybir.dt.float32
BF16 = mybir.dt.bfloat16
F8 = mybir.dt.float8e4
AF = mybir.ActivationFunctionType
DR = mybir.MatmulPerfMode.DoubleRow

_CACHE = {}


def _build_nc():
    nc = bacc.Bacc(None, target_bir_lowering=False, debug=False)

    # ---- external inputs (host-retiled [P, i, n]: row (i*P+p) -> [p, i])
    statt = nc.dram_tensor("statt", [P, NT_I, KN], F8, kind="ExternalInput")
    wekt = nc.dram_tensor("wekt", [P, NT_I, P], F8, kind="ExternalInput")
    wevt = nc.dram_tensor("wevt", [P, NT_I, P], F8, kind="ExternalInput")
    xbt = nc.dram_tensor("xbt", [P, NT_I, BL], BF16, kind="ExternalInput")
    wk8t = nc.dram_tensor("wk8t", [P, NT_I, C], F8, kind="ExternalInput")
    wvbt = nc.dram_tensor("wvbt", [P, NT_I, C], BF16, kind="ExternalInput")
    bekc = nc.dram_tensor("bekc", [P, 1], F32, kind="ExternalInput")
    bevc = nc.dram_tensor("bevc", [P, 1], F32, kind="ExternalInput")
    wwc = nc.dram_tensor("wwc", [P, 1], BF16, kind="ExternalInput")
    bkv = nc.dram_tensor("bkv", [P, NT_KV], F32, kind="ExternalInput")
    id8 = nc.dram_tensor("id8", [P, P], BF16, kind="ExternalInput")
    wout = nc.dram_tensor("wout", [P, NT_KV * K], BF16, kind="ExternalInput")
    bwh = nc.dram_tensor("bwh", [1, 1], F32, kind="ExternalInput")
    boutt = nc.dram_tensor("boutt", [K, 1], F32, kind="ExternalInput")
    outT = nc.dram_tensor("outT", [K, BL], F32, kind="ExternalOutput")

    # ---- collective buffers
    ccg_in = nc.dram_tensor("ccg_in", [P, KN], F8)
    ccg_out = nc.dram_tensor("ccg_out", [NCORES, P, KN], F8,
                             addr_space="Shared")
    ccv_in = nc.dram_tensor("ccv_in", [P, KN], F8)
    ccv_out = nc.dram_tensor("ccv_out", [NCORES, P, NT_KN, P], F8,
                             addr_space="Shared")
    ccr_sq_in = nc.dram_tensor("ccr_sq_in", [1, KN], F32)
    ccr_sq_out = nc.dram_tensor("ccr_sq_out", [1, KN], F32)
    ccr_rw_in = nc.dram_tensor("ccr_rw_in", [1, KN], F32)
    ccr_rw_out = nc.dram_tensor("ccr_rw_out", [1, KN], F32)
    GROUPS = [list(range(NCORES))]

    tc_cm = tile.TileContext(nc)
    tc = tc_cm.__enter__()

    # ================= singles (no mid-program frees needed) =========
    frees = []

    def single(shape, dt, name):
        t, f = tc.tile(shape, dt, name=name)
        frees.append(f)
        return t

    warm = single([1, 16], F32, "warm")
    warmo = single([1, 16], F32, "warmo")
    epsb = single([1, 1], F32, "epsb")
    epsb_p = single([P, 1], F32, "epsb_p")
    ones_col = single([P, 1], BF16, "ones_col")
    s16_row = single([1, P], BF16, "s16_row")
    h64_row = single([1, P], BF16, "h64_row")
    bkv_sb = single([P, NT_KV], F32, "bkv_sb")
    bwh_sb = single([1, 1], F32, "bwh_sb")
    bout_sb = single([K, 1], F32, "bout_sb")
    wo_sb = single([P, NT_KV * K], BF16, "wo_sb")

    fr_all = single([P, NT_C, BL], BF16, "fr_all")
    vr_all = single([P, NT_C, BL], BF16, "vr_all")
    wf_all = single([P, NT_KN, BL], F8, "wf_all")
    kn_all = single([P, NT_C, BL], F8, "kn_all")
    eksl = single([P, NT_C, KN], F8, "eksl")
    # gate stationary: col 0 = ones (gse), col 32 = wEv row (gtg); the
    # 33-wide output puts gtg at partition 32, a legal DVE base.
    lhs2 = single([P, NT_KN, 33], BF16, "lhs2")
    inv_col = single([P, NT_KN], F32, "inv_col")
    sq_col = single([P, NT_KN], F32, "sq_col")
    wv_col = single([P, NT_KN], F32, "wv_col")
    nrm_sb = single([P, NT_KN], F32, "nrm_sb")
    kT_all = single([P, NT_C, BL], BF16, "kT_all")
    wvb_sb = single([P, NT_I, C], BF16, "wvb_sb")
    xb_sb = single([P, NT_I, BL], BF16, "xb_sb")
    wk8_sb = single([P, NT_I, C], F8, "wk8_sb")
    x8_sb = single([P, NT_I, BL], F8, "x8_sb")
    wek_sb = single([P, NT_I, P], F8, "wek_sb")
    wev_sb = single([P, NT_I, P], F8, "wev_sb")
    bekc_sb = single([P, 1], F32, "bekc_sb")
    bevc_sb = single([P, 1], F32, "bevc_sb")
    wwc_sb = single([P, 1], BF16, "wwc_sb")
    id8_sb = single([P, P], BF16, "id8_sb")
    sts = [single([P, NT_I, 512], F8, f"st{c}") for c in range(NCH)]

    # ================= memsets (vector queue) ========================
    nc.vector.memset(warm[:], 1.0)
    nc.vector.memset(epsb[:], EPS * EPS)
    nc.vector.memset(epsb_p[:], 256.0 * EPS * EPS)
    nc.vector.memset(ones_col[:], 1.0)
    nc.vector.memset(s16_row[:], KNS)
    nc.vector.memset(h64_row[:], WFS / 2)
    nc.vector.memset(lhs2[:], 0.0)
    nc.vector.memset(lhs2[:, :, 0], 1.0)

    # ================= upfront DMA issues ============================
    # HBM saturates ~380GB/s with all rings pulling, so priority order
    # matters: stat (P-phase, feeds the collective chain) spread over
    # all three queues first, then xb/wk8 (KV), then wvb.  The scalar
    # engine issues exactly 4 head-of-queue loads and NO trigger after
    # a collective, ever: a post-collective HWDGE trigger blocks the
    # issuing ENGINE until the collective completes.  All compute-output
    # DMAs ride gpsimd SW-DGE.  x8 is cast from xb on DVE (exact).
    nc.sync.dma_start(bkv_sb[:], bkv[:])
    nc.sync.dma_start(bekc_sb[:], bekc[:])
    nc.sync.dma_start(bevc_sb[:], bevc[:])
    nc.sync.dma_start(wek_sb[:], wekt[:])
    nc.scalar.dma_start(sts[1][:], statt[:, :, 512:1024])
    nc.scalar.dma_start(xb_sb[:], xbt[:])
    nc.scalar.dma_start(wk8_sb[:, 0:NT_I // 2, :], wk8t[:, 0:NT_I // 2, :])
    nc.scalar.dma_start(wk8_sb[:, NT_I // 2:, :], wk8t[:, NT_I // 2:, :])
    # tiny Sqrt loads the sqrt_and_others table early, off critical path
    nc.scalar.activation(warmo[:], warm[:], AF.Sqrt)
    nc.sync.dma_start(sts[0][:], statt[:, :, 0:512])
    nc.gpsimd.dma_start(sts[2][:], statt[:, :, 2 * 512:3 * 512])
    nc.sync.dma_start(sts[3][:], statt[:, :, 3 * 512:4 * 512])
    nc.gpsimd.dma_start(sts[4][:], statt[:, :, 4 * 512:5 * 512])
    nc.gpsimd.dma_start(wev_sb[:], wevt[:])
    nc.gpsimd.dma_start(wwc_sb[:], wwc[:])
    nc.gpsimd.dma_start(id8_sb[:], id8[:])
    nc.gpsimd.dma_start(bwh_sb[:], bwh[:])
    nc.gpsimd.dma_start(bout_sb[:], boutt[:])
    nc.gpsimd.dma_start(wo_sb[:], wout[:])
    for m in range(4):
        ms = slice(m * P, (m + 1) * P)
        nc.sync.dma_start(wvb_sb[:, :, ms], wvbt[:, :, ms])

    # ================= Phase P: Ek pass, then Ev pass ================
    with tc.tile_pool(name="pw", bufs=3) as pw, \
         tc.tile_pool(name="rowp", bufs=2) as rowp, \
         tc.tile_pool(name="ppk", bufs=2, space="PSUM") as ppk, \
         tc.tile_pool(name="prow", bufs=2, space="PSUM") as prow, \
         tc.tile_pool(name="ptp", bufs=2, space="PSUM") as ptp:
        # ---- Ek chunks
        for ch in range(NCH):
            cs = slice(ch * 512, (ch + 1) * 512)
            ek_ps = ppk.tile([P, 512], F32, tag="ek")
            for i2 in range(NT_I // 2):
                nc.tensor.matmul(ek_ps[:], wek_sb[:, 2 * i2:2 * i2 + 2, :],
                                 sts[ch][:, 2 * i2:2 * i2 + 2, :],
                                 start=(i2 == 0), stop=(i2 == NT_I // 2 - 1),
                                 perf_mode=DR)
            ek8 = pw.tile([P, 512], F8, tag="ek8")
            nc.scalar.activation(ek8[:], ek_ps[:], AF.Identity,
                                 bias=bekc_sb[:], scale=1.0 / WS)
            nc.gpsimd.dma_start(ccg_in[:, cs], ek8[:])
            sqt = pw.tile([P, 512], BF16, tag="sqt")
            nc.scalar.activation(sqt[:], ek_ps[:], AF.Square,
                                 bias=bekc_sb[:], scale=1.0 / WS)
            sq_ps = prow.tile([1, 512], F32, tag="row")
            nc.tensor.matmul(sq_ps[:], ones_col[:], sqt[:])
            sq_row = rowp.tile([1, 512], F32, tag="sqr")
            nc.vector.tensor_copy(sq_row[:], sq_ps[:])
            nc.gpsimd.dma_start(ccr_sq_in[0:1, cs], sq_row[:])

        # x8 = fp8 cast of xb (exact; DVE) — xb has landed by now
        with nc.allow_low_precision(reason="fp8 x for the k projection"):
            for i in range(NT_I):
                nc.vector.tensor_copy(x8_sb[:, i, :], xb_sb[:, i, :])

        # remaining v-half weight slices (after the P-Ek outs so those
        # reach HBM first — the collectives need them)
        for m in range(4, NT_C):
            ms = slice(m * P, (m + 1) * P)
            nc.gpsimd.dma_start(wvb_sb[:, :, ms], wvbt[:, :, ms])

        # ---- collectives round 1 (sq AllReduce + Ek AllGather)
        nc.gpsimd.collective_compute(
            "AllReduce", mybir.AluOpType.add, replica_groups=GROUPS,
            ins=[ccr_sq_in[:].opt()], outs=[ccr_sq_out[:].opt()])
        nc.gpsimd.collective_compute(
            "AllGather", mybir.AluOpType.bypass, replica_groups=GROUPS,
            ins=[ccg_in[:].opt()], outs=[ccg_out[:].opt()])
        nc.gpsimd.dma_start(
            sq_col[:], ccr_sq_out[0, :].rearrange("(j p) -> p j", p=P))

        # ---- Ev chunks (stat already resident; no input DMA here)
        for ch in range(NCH):
            cs = slice(ch * 512, (ch + 1) * 512)
            ev_ps = ppk.tile([P, 512], F32, tag="ev")
            for i2 in range(NT_I // 2):
                nc.tensor.matmul(ev_ps[:], wev_sb[:, 2 * i2:2 * i2 + 2, :],
                                 sts[ch][:, 2 * i2:2 * i2 + 2, :],
                                 start=(i2 == 0), stop=(i2 == NT_I // 2 - 1),
                                 perf_mode=DR)
            evbf = pw.tile([P, 512], BF16, tag="evbf")
            nc.scalar.activation(evbf[:], ev_ps[:], AF.Identity,
                                 bias=bevc_sb[:], scale=1.0 / WS)
            rw_ps = prow.tile([1, 512], F32, tag="row")
            nc.tensor.matmul(rw_ps[:], wwc_sb[:], evbf[:])
            rw_row = rowp.tile([1, 512], F32, tag="rwr")
            nc.vector.tensor_copy(rw_row[:], rw_ps[:])
            nc.gpsimd.dma_start(ccr_rw_in[0:1, cs], rw_row[:])
            tp_ps = ptp.tile([P, 512], BF16, tag="tp")
            for q in range(4):
                nc.tensor.transpose(tp_ps[:, q * P:(q + 1) * P],
                                    evbf[:, q * P:(q + 1) * P], id8_sb[:])
            evt8 = pw.tile([P, 512], F8, tag="evt8")
            nc.scalar.copy(evt8[:], tp_ps[:])
            nc.gpsimd.dma_start(ccv_in[:, cs], evt8[:])

        # ---- eksl scatter loads: 4 on the sync ring (its engine can
        # afford to block on the AG-ek wait), 4 on gpsimd after the
        # round-2 triggers.  NEVER on scalar: the trigger's collective
        # wait would stall every KV activation behind it.
        for m in range(4):
            nc.sync.dma_start(eksl[:, m, :], ccg_out[m])

        # ---- collectives round 2 (row AllReduce + Ev AllGather)
        nc.gpsimd.collective_compute(
            "AllReduce", mybir.AluOpType.add, replica_groups=GROUPS,
            ins=[ccr_rw_in[:].opt()], outs=[ccr_rw_out[:].opt()])
        nc.gpsimd.dma_start(
            wv_col[:], ccr_rw_out[0, :].rearrange("(j p) -> p j", p=P))
        nc.gpsimd.collective_compute(
            "AllGather", mybir.AluOpType.bypass, replica_groups=GROUPS,
            ins=[ccv_in[:].opt()], outs=[ccv_out[:].opt()])
        for m in range(4, NT_C):
            nc.gpsimd.dma_start(eksl[:, m, :], ccg_out[m])

    # ================= Phase KV ======================================
    with tc.tile_pool(name="pkv", bufs=3, space="PSUM") as pkv, \
         tc.tile_pool(name="pssk", bufs=1, space="PSUM") as pssk, \
         tc.tile_pool(name="pbc", bufs=1, space="PSUM") as pbc, \
         tc.tile_pool(name="sqp", bufs=2) as sqp, \
         tc.tile_pool(name="kvw", bufs=2) as kvw:
        # ---- k half (fp8 DoubleRow) + sum-of-squares
        ssk = pssk.tile([1, BL], F32)
        for m in range(NT_C):
            kv_ps = pkv.tile([P, BL], F32, tag="kv", name=f"k{m}")
            for i2 in range(NT_I // 2):
                nc.tensor.matmul(
                    kv_ps[:], wk8_sb[:, 2 * i2:2 * i2 + 2, m * P:(m + 1) * P],
                    x8_sb[:, 2 * i2:2 * i2 + 2, :],
                    start=(i2 == 0), stop=(i2 == NT_I // 2 - 1),
                    perf_mode=DR)
            nc.scalar.activation(kT_all[:, m, :], kv_ps[:], AF.Identity,
                                 bias=bkv_sb[:, m:m + 1], scale=1.0 / WS)
            sqk = sqp.tile([P, BL], BF16, tag="sqk")
            nc.scalar.activation(sqk[:], kv_ps[:], AF.Square,
                                 bias=bkv_sb[:, m:m + 1], scale=1.0 / WS)
            nc.tensor.matmul(ssk[:], ones_col[:], sqk[:],
                             start=(m == 0), stop=(m == NT_C - 1))
        # ---- norms: both Sqrts together (table 3), then warm table 0
        nk = kvw.tile([1, BL], F32, tag="nk")
        nc.scalar.activation(nk[:], ssk[:], AF.Sqrt, bias=epsb[:])
        nc.scalar.activation(nrm_sb[:], sq_col[:], AF.Sqrt,
                             bias=epsb_p[:], scale=256.0)
        nc.scalar.activation(warmo[:], warm[:], AF.Exp)
        invk = kvw.tile([1, BL], BF16, tag="invk")
        with nc.allow_low_precision(reason="bf16 1/|k| feeds fp8 kn"):
            nc.vector.reciprocal(invk[:], nk[:])
        nc.vector.reciprocal(inv_col[:], nrm_sb[:])
        # ---- v half (bf16), bc broadcast inserted after 2 tiles
        for m in range(NT_C):
            kv_ps = pkv.tile([P, BL], F32, tag="kv", name=f"v{m}")
            for i in range(NT_I):
                nc.tensor.matmul(
                    kv_ps[:], wvb_sb[:, i, m * P:(m + 1) * P],
                    xb_sb[:, i, :],
                    start=(i == 0), stop=(i == NT_I - 1))
            nc.scalar.activation(vr_all[:, m, :], kv_ps[:], AF.Relu,
                                 bias=bkv_sb[:, NT_C + m:NT_C + m + 1])
            if m == 1:
                bc = pbc.tile([P, BL], F32)
                nc.tensor.matmul(bc[:], s16_row[:], invk[:])
                for mm in range(NT_C):
                    nc.vector.tensor_mul(kn_all[:, mm, :],
                                         kT_all[:, mm, :], bc[:])

    # lhs2 col 32 = wEv gate row (col 0 stays the memset 1.0)
    nc.vector.tensor_copy(lhs2[:, :, 32], wv_col[:])

    # ================= Fused SIM + GATE + WF =========================
    with tc.tile_pool(name="gw", bufs=3) as gw, \
         tc.tile_pool(name="esw", bufs=9) as esw, \
         tc.tile_pool(name="psim", bufs=3, space="PSUM") as psim, \
         tc.tile_pool(name="pg", bufs=2, space="PSUM") as pg, \
         tc.tile_pool(name="pbc2", bufs=2, space="PSUM") as pbc2:
        es_hist = {}
        sk_hist = {}

        def emit_wf(kk):
            # deferred one iteration: bcs broadcast + wf multiplies for
            # prototype kk (gate chain long since finished -> no PE stall)
            bcs = pbc2.tile([P, BL], F32, tag="bcs")
            nc.tensor.matmul(bcs[:], h64_row[:], sk_hist[kk][:])
            for j in range(TPK):
                nc.vector.tensor_mul(wf_all[:, kk * TPK + j, :],
                                     es_hist[(kk, j)][:], bcs[:])

        for k in range(K):
            ks0 = k * 512
            gg = pg.tile([33, BL], F32, tag="gg")
            for j in range(TPK):
                kt = k * TPK + j
                ps = psim.tile([P, BL], F32, tag="sim")
                for m2 in range(NT_C // 2):
                    nc.tensor.matmul(
                        ps[:],
                        eksl[:, 2 * m2:2 * m2 + 2,
                             ks0 + j * P:ks0 + (j + 1) * P],
                        kn_all[:, 2 * m2:2 * m2 + 2, :],
                        start=(m2 == 0), stop=(m2 == NT_C // 2 - 1),
                        perf_mode=DR)
                es = esw.tile([P, BL], BF16, tag="es", name=f"es{kt}")
                nc.scalar.activation(es[:], ps[:], AF.Exp,
                                     scale=inv_col[:, kt:kt + 1])
                es_hist[(k, j)] = es
                nc.tensor.matmul(gg[:], lhs2[:, kt, 0:33], es[:],
                                 start=(j == 0), stop=(j == TPK - 1))
            # gate chain: rs=1/sum, tg=gtg*rs, sigmoid via tanh,
            # sk = (1+t)*rs (*64 folded into the broadcast row)
            rs = gw.tile([1, BL], F32, tag="rs")
            nc.vector.reciprocal(rs[:], gg[0:1, :])
            tg = gw.tile([1, BL], F32, tag="tg")
            nc.vector.tensor_mul(tg[:], gg[32:33, :], rs[:])
            tt = gw.tile([1, BL], F32, tag="tt")
            nc.scalar.activation(tt[:], tg[:], AF.Tanh,
                                 scale=0.5, bias=bwh_sb[0:1, 0:1])
            sk2 = gw.tile([1, BL], BF16, tag="sk2")
            nc.vector.scalar_tensor_tensor(
                sk2[:], tt[:], 1.0, rs[:],
                op0=mybir.AluOpType.add, op1=mybir.AluOpType.mult)
            sk_hist[k] = sk2
            if k > 0:
                emit_wf(k - 1)
        emit_wf(K - 1)

    # ================= Phase FE + OUT ================================
    with tc.tile_pool(name="evp", bufs=3) as evp, \
         tc.tile_pool(name="ow", bufs=1) as ow, \
         tc.tile_pool(name="pfe", bufs=3, space="PSUM") as pfe, \
         tc.tile_pool(name="pout", bufs=1, space="PSUM") as pout:
        po = pout.tile([K, BL], F32)
        for j in range(NT_C):
            nc.tensor.matmul(po[:], wo_sb[:, j * K:(j + 1) * K],
                             vr_all[:, j, :],
                             start=(j == 0), stop=False)
        evtts = {}
        for mc in range(2):
            evtt = evp.tile([P, NT_KN, P], F8, tag="evt", name=f"ev{mc}")
            eng = nc.sync if mc % 2 == 0 else nc.gpsimd
            eng.dma_start(evtt[:], ccv_out[mc])
            evtts[mc] = evtt
        for mc in range(NT_C):
            evtt = evtts.pop(mc)
            if mc + 2 < NT_C:
                nxt = evp.tile([P, NT_KN, P], F8, tag="evt",
                               name=f"ev{mc + 2}")
                eng = nc.sync if mc % 2 == 0 else nc.gpsimd
                eng.dma_start(nxt[:], ccv_out[mc + 2])
                evtts[mc + 2] = nxt
            ps = pfe.tile([P, BL], F32, tag="feps")
            for t2 in range(NT_KN // 2):
                nc.tensor.matmul(
                    ps[:], evtt[:, 2 * t2:2 * t2 + 2, :],
                    wf_all[:, 2 * t2:2 * t2 + 2, :],
                    start=(t2 == 0), stop=(t2 == NT_KN // 2 - 1),
                    perf_mode=DR)
            nc.scalar.activation(fr_all[:, mc, :], ps[:], AF.Relu,
                                 scale=1.0 / WFS)
            nc.tensor.matmul(po[:], wo_sb[:, (NT_C + mc) * K:
                                          (NT_C + mc + 1) * K],
                             fr_all[:, mc, :],
                             start=False, stop=(mc == NT_C - 1))
        osb = ow.tile([K, BL], F32)
        nc.scalar.activation(osb[:], po[:], AF.Identity, bias=bout_sb[:])
        nc.sync.dma_start(outT[:], osb[:])

    for f in reversed(frees):
        f()

    tc_cm.__exit__(None, None, None)
    nc.compile()
    return nc


def _tile_rows(a):
    """[NT_I*P, n] -> [P, NT_I, n]: row (i*P + p) -> [p, i]."""
    n = a.shape[1]
    return np.ascontiguousarray(
        a.reshape(NT_I, P, n).transpose(1, 0, 2))


def _f8(a):
    return np.clip(a, -240.0, 240.0).astype(ml_dtypes.float8_e4m3)


def _host_prep(inputs):
    bf = ml_dtypes.bfloat16
    x_last = np.asarray(inputs["x"])[:, -1, :]  # [B, CH] f32
    wekT = np.asarray(inputs["WEk"]).T * WS  # [CH, C]
    wevT = np.asarray(inputs["WEv"]).T * WS
    shared = {
        "wk8t": _f8(_tile_rows(np.asarray(inputs["Wk"]).T * WS)),
        "wvbt": _tile_rows(np.asarray(inputs["Wv"]).T).astype(bf),
        "statt": _f8(_tile_rows(
            np.asarray(inputs["static"]).transpose(1, 0, 2).reshape(CH, KN))),
        "bkv": np.ascontiguousarray(
            np.concatenate([inputs["bk"], inputs["bv"]]).reshape(NT_KV, P).T),
        "id8": np.eye(P, dtype=bf),
        "wout": np.ascontiguousarray(
            np.asarray(inputs["Wout"]).T.reshape(NT_KV, P, K)
            .transpose(1, 0, 2).reshape(P, NT_KV * K)).astype(bf),
        "bwh": np.asarray(inputs["bw"], dtype=np.float32).reshape(1, 1) * 0.5,
        "boutt": np.asarray(inputs["bout"], dtype=np.float32).reshape(K, 1),
    }
    in_maps = []
    for r in range(NCORES):
        cslc = slice(r * P, (r + 1) * P)
        m = dict(shared)
        xs = np.ascontiguousarray(x_last[r * BL:(r + 1) * BL].T)
        m["xbt"] = _tile_rows(xs).astype(bf)
        m["wekt"] = _f8(_tile_rows(np.ascontiguousarray(wekT[:, cslc])))
        m["wevt"] = _f8(_tile_rows(np.ascontiguousarray(wevT[:, cslc])))
        m["bekc"] = np.ascontiguousarray(
            np.asarray(inputs["bEk"], dtype=np.float32)[cslc].reshape(P, 1))
        m["bevc"] = np.ascontiguousarray(
            np.asarray(inputs["bEv"], dtype=np.float32)[cslc].reshape(P, 1))
        m["wwc"] = np.ascontiguousarray(
            np.asarray(inputs["Ww"])[0, cslc].reshape(P, 1)).astype(bf)
        in_maps.append(m)
    return in_maps


def kernel(**inputs):
    if "nc" not in _CACHE:
        _CACHE["nc"] = _build_nc()
    nc = _CACHE["nc"]
    in_maps = _host_prep(inputs)
    res = bass_utils.run_bass_kernel_spmd(
        nc, in_maps, core_ids=list(range(NCORES)), trace=False)
    out = np.concatenate(
        [res.results[r]["outT"].T for r in range(NCORES)], axis=0)
    return np.ascontiguousarray(out[:, :, None], dtype=np.float32)


# revision 26
# speedup vs baseline: 1.0981x; 1.0981x over previous
"""Trainium2 Bass kernel for nn_Colar_static (retrieval_knn).

Data-parallel over batch B across 8 cores + tensor-parallel split of
the Ek/Ev prototype projections over C=1024 (each core a [128, K*N]
slab), slabs exchanged with on-chip collectives.

v2 changes vs the 304us baseline:
  - fp8 (e4m3) DoubleRow matmuls (0.5 cyc/row) for the Ek/Ev slab
    projections, the K projection, SIM and FE. Weights host-scaled by
    32 (un-scaled via activation `scale`); kn scaled by 16, wf by 128.
    V projection + out projection stay bf16: the v-half dominates the
    output (out std 0.425 of 0.432) so its precision gates accuracy,
    while everything downstream of the softmax is attenuated ~30x.
  - exactly 2 activation-table loads: sqrt_and_others for P/KV
    (Identity/Square/Relu/Sqrt), exp_and_others for SIM/FE/OUT
    (Exp/Tanh/Identity/Copy/Relu). sigmoid -> 0.5+0.5*tanh(x/2),
    1/sum -> DVE reciprocal. Table switch is warmed mid-KV.
  - Ek AllGather issued right after the Ek pass (all other bulk HWDGE
    pre-issued; stat/x/w all SBUF-resident in fp8), fp8 gather payloads.
  - gse/gtg merged into one [P,2]-stationary matmul; bcs broadcast
    deferred one k-iteration so PE never stalls on the gate chain;
    wf/kn multiplies read PSUM broadcasts directly on DVE.
"""

import sys

for _p in ("/opt/trn_rl_repo", "/opt/pypackages"):
    if _p not in sys.path:
        sys.path.append(_p)

import numpy as np
import ml_dtypes

import concourse.bass as bass
import concourse.mybir as mybir
import concourse.tile as tile
from concourse import bacc
from concourse import bass_utils

B, T, CH, C, N, K = 4096, 8, 2048, 1024, 512, 5
NCORES = 8
BL = B // NCORES            # 512 batch rows per core
KN = K * N                  # 2560 prototype columns
P = 128
NT_I = CH // P              # 16 contraction tiles (input channels)
NT_C = C // P               # 8 tiles over C
NT_KN = KN // P             # 20 tiles over K*N
NT_KV = 2 * C // P          # 16 tiles over [k|v] output channels
TPK = NT_KN // K            # 4 kn-tiles per prototype
NCH = KN // 512             # 5 column chunks for the slab projections
EPS = 1e-8
WS = 32.0                   # fp8 weight prescale
KNS = 16.0                  # kn prescale
WFS = 128.0                 # wf prescale

F32 = mybir.dt.float32
BF16 = mybir.dt.bfloat16
F8 = mybir.dt.float8e4
AF = mybir.ActivationFunctionType
DR = mybir.MatmulPerfMode.DoubleRow

_CACHE = {}


def _build_nc():
    nc = bacc.Bacc(None, target_bir_lowering=False, debug=False)

    # ---- external inputs (host-retiled [P, i, n]: row (i*P+p) -> [p, i])
    # chunk-major / m-major layouts: every DMA below reads DRAM fully
    # contiguously (512B-burst strided reads run at ~50GB/s, 1/4 rate)
    statt = nc.dram_tensor("statt", [NCH, P, NT_I, 512], F8,
                           kind="ExternalInput")
    wekt = nc.dram_tensor("wekt", [P, NT_I, P], F8, kind="ExternalInput")
    wevt = nc.dram_tensor("wevt", [P, NT_I, P], F8, kind="ExternalInput")
    xbt = nc.dram_tensor("xbt", [P, NT_I, BL], BF16, kind="ExternalInput")
    wk8t = nc.dram_tensor("wk8t", [P, NT_I, C], F8, kind="ExternalInput")
    wvbt = nc.dram_tensor("wvbt", [NT_C, P, NT_I, P], BF16,
                          kind="ExternalInput")
    bekc = nc.dram_tensor("bekc", [P, 1], F32, kind="ExternalInput")
    bevc = nc.dram_tensor("bevc", [P, 1], F32, kind="ExternalInput")
    wwc = nc.dram_tensor("wwc", [P, 1], BF16, kind="ExternalInput")
    bkv = nc.dram_tensor("bkv", [P, NT_KV], F32, kind="ExternalInput")
    id8 = nc.dram_tensor("id8", [P, P], BF16, kind="ExternalInput")
    wout = nc.dram_tensor("wout", [P, NT_KV * K], BF16, kind="ExternalInput")
    bwh = nc.dram_tensor("bwh", [1, 1], F32, kind="ExternalInput")
    boutt = nc.dram_tensor("boutt", [K, 1], F32, kind="ExternalInput")
    outT = nc.dram_tensor("outT", [K, BL], F32, kind="ExternalOutput")

    # ---- collective buffers
    ccg_in = nc.dram_tensor("ccg_in", [P, KN], F8)
    ccg_out = nc.dram_tensor("ccg_out", [NCORES, P, KN], F8,
                             addr_space="Shared")
    ccv_in = nc.dram_tensor("ccv_in", [P, KN], F8)
    ccv_out = nc.dram_tensor("ccv_out", [NCORES, P, NT_KN, P], F8,
                             addr_space="Shared")
    ccr_sq_in = nc.dram_tensor("ccr_sq_in", [1, KN], F32)
    ccr_sq_out = nc.dram_tensor("ccr_sq_out", [1, KN], F32)
    ccr_rw_in = nc.dram_tensor("ccr_rw_in", [1, KN], F32)
    ccr_rw_out = nc.dram_tensor("ccr_rw_out", [1, KN], F32)
    GROUPS = [list(range(NCORES))]

    tc_cm = tile.TileContext(nc)
    tc = tc_cm.__enter__()

    # ================= singles (no mid-program frees needed) =========
    frees = []

    def single(shape, dt, name):
        t, f = tc.tile(shape, dt, name=name)
        frees.append(f)
        return t

    warm = single([1, 16], F32, "warm")
    warmo = single([1, 16], F32, "warmo")
    epsb = single([1, 1], F32, "epsb")
    epsb_p = single([P, 1], F32, "epsb_p")
    ones_col = single([P, 1], BF16, "ones_col")
    s16_row = single([1, P], BF16, "s16_row")
    h64_row = single([1, P], BF16, "h64_row")
    bkv_sb = single([P, NT_KV], F32, "bkv_sb")
    bwh_sb = single([1, 1], F32, "bwh_sb")
    bout_sb = single([K, 1], F32, "bout_sb")
    wo_sb = single([P, NT_KV * K], BF16, "wo_sb")

    fr_all = single([P, NT_C, BL], BF16, "fr_all")
    vr_all = single([P, NT_C, BL], BF16, "vr_all")
    wf_all = single([P, NT_KN, BL], F8, "wf_all")
    kn_all = single([P, NT_C, BL], F8, "kn_all")
    eksl = single([P, NT_C, KN], F8, "eksl")
    # gate stationary: col 0 = ones (gse), col 32 = wEv row (gtg); the
    # 33-wide output puts gtg at partition 32, a legal DVE base.
    lhs2 = single([P, NT_KN, 33], BF16, "lhs2")
    inv_col = single([P, NT_KN], F32, "inv_col")
    sq_col = single([P, NT_KN], F32, "sq_col")
    wv_col = single([P, NT_KN], F32, "wv_col")
    nrm_sb = single([P, NT_KN], F32, "nrm_sb")
    kT_all = single([P, NT_C, BL], BF16, "kT_all")
    wvb_sb = single([P, NT_I, C], BF16, "wvb_sb")
    xb_sb = single([P, NT_I, BL], BF16, "xb_sb")
    wk8_sb = single([P, NT_I, C], F8, "wk8_sb")
    x8_sb = single([P, NT_I, BL], F8, "x8_sb")
    wek_sb = single([P, NT_I, P], F8, "wek_sb")
    wev_sb = single([P, NT_I, P], F8, "wev_sb")
    bekc_sb = single([P, 1], F32, "bekc_sb")
    bevc_sb = single([P, 1], F32, "bevc_sb")
    wwc_sb = single([P, 1], BF16, "wwc_sb")
    id8_sb = single([P, P], BF16, "id8_sb")
    sts = [single([P, NT_I, 512], F8, f"st{c}") for c in range(NCH)]

    # ================= memsets (vector queue) ========================
    nc.vector.memset(warm[:], 1.0)
    nc.vector.memset(epsb[:], EPS * EPS)
    nc.vector.memset(epsb_p[:], 256.0 * EPS * EPS)
    nc.vector.memset(ones_col[:], 1.0)
    nc.vector.memset(s16_row[:], KNS)
    nc.vector.memset(h64_row[:], WFS / 2)
    nc.vector.memset(lhs2[:], 0.0)
    nc.vector.memset(lhs2[:, :, 0], 1.0)

    # ================= upfront DMA issues ============================
    # HBM saturates ~380GB/s with all rings pulling, so priority order
    # matters: stat (P-phase, feeds the collective chain) spread over
    # all three queues first, then xb/wk8 (KV), then wvb.  The scalar
    # engine issues exactly 4 head-of-queue loads and NO trigger after
    # a collective, ever: a post-collective HWDGE trigger blocks the
    # issuing ENGINE until the collective completes.  All compute-output
    # DMAs ride gpsimd SW-DGE.  x8 is cast from xb on DVE (exact).
    nc.sync.dma_start(bkv_sb[:], bkv[:])
    nc.sync.dma_start(bekc_sb[:], bekc[:])
    nc.sync.dma_start(bevc_sb[:], bevc[:])
    nc.sync.dma_start(wek_sb[:], wekt[:])
    nc.scalar.dma_start(sts[1][:], statt[1])
    nc.scalar.dma_start(xb_sb[:], xbt[:])
    nc.scalar.dma_start(wk8_sb[:, 0:NT_I // 2, :], wk8t[:, 0:NT_I // 2, :])
    nc.scalar.dma_start(wk8_sb[:, NT_I // 2:, :], wk8t[:, NT_I // 2:, :])
    # tiny Sqrt loads the sqrt_and_others table early, off critical path
    nc.scalar.activation(warmo[:], warm[:], AF.Sqrt)
    nc.sync.dma_start(sts[0][:], statt[0])
    nc.gpsimd.dma_start(sts[2][:], statt[2])
    nc.sync.dma_start(sts[3][:], statt[3])
    nc.gpsimd.dma_start(sts[4][:], statt[4])
    nc.gpsimd.dma_start(wev_sb[:], wevt[:])
    nc.gpsimd.dma_start(wwc_sb[:], wwc[:])
    nc.gpsimd.dma_start(id8_sb[:], id8[:])
    nc.gpsimd.dma_start(bwh_sb[:], bwh[:])
    nc.gpsimd.dma_start(bout_sb[:], boutt[:])
    nc.gpsimd.dma_start(wo_sb[:], wout[:])
    for m in range(4):
        ms = slice(m * P, (m + 1) * P)
        nc.sync.dma_start(wvb_sb[:, :, ms], wvbt[m])

    # ================= Phase P: Ek pass, then Ev pass ================
    with tc.tile_pool(name="pw", bufs=3) as pw, \
         tc.tile_pool(name="rowp", bufs=2) as rowp, \
         tc.tile_pool(name="ppk", bufs=2, space="PSUM") as ppk, \
         tc.tile_pool(name="prow", bufs=2, space="PSUM") as prow, \
         tc.tile_pool(name="ptp", bufs=2, space="PSUM") as ptp:
        # ---- Ek chunks
        for ch in range(NCH):
            cs = slice(ch * 512, (ch + 1) * 512)
            ek_ps = ppk.tile([P, 512], F32, tag="ek")
            for i2 in range(NT_I // 2):
                nc.tensor.matmul(ek_ps[:], wek_sb[:, 2 * i2:2 * i2 + 2, :],
                                 sts[ch][:, 2 * i2:2 * i2 + 2, :],
                                 start=(i2 == 0), stop=(i2 == NT_I // 2 - 1),
                                 perf_mode=DR)
            ek8 = pw.tile([P, 512], F8, tag="ek8")
            nc.scalar.activation(ek8[:], ek_ps[:], AF.Identity,
                                 bias=bekc_sb[:], scale=1.0 / WS)
            nc.gpsimd.dma_start(ccg_in[:, cs], ek8[:])
            sqt = pw.tile([P, 512], BF16, tag="sqt")
            nc.scalar.activation(sqt[:], ek_ps[:], AF.Square,
                                 bias=bekc_sb[:], scale=1.0 / WS)
            sq_ps = prow.tile([1, 512], F32, tag="row")
            nc.tensor.matmul(sq_ps[:], ones_col[:], sqt[:])
            sq_row = rowp.tile([1, 512], F32, tag="sqr")
            nc.vector.tensor_copy(sq_row[:], sq_ps[:])
            nc.gpsimd.dma_start(ccr_sq_in[0:1, cs], sq_row[:])

        # x8 = fp8 cast of xb (exact; DVE) — xb has landed by now
        with nc.allow_low_precision(reason="fp8 x for the k projection"):
            for i in range(NT_I):
                nc.vector.tensor_copy(x8_sb[:, i, :], xb_sb[:, i, :])

        # remaining v-half weight slices (after the P-Ek outs so those
        # reach HBM first — the collectives need them)
        for m in range(4, NT_C):
            ms = slice(m * P, (m + 1) * P)
            nc.gpsimd.dma_start(wvb_sb[:, :, ms], wvbt[m])

        # ---- collectives round 1 (sq AllReduce + Ek AllGather)
        nc.gpsimd.collective_compute(
            "AllReduce", mybir.AluOpType.add, replica_groups=GROUPS,
            ins=[ccr_sq_in[:].opt()], outs=[ccr_sq_out[:].opt()])
        nc.gpsimd.collective_compute(
            "AllGather", mybir.AluOpType.bypass, replica_groups=GROUPS,
            ins=[ccg_in[:].opt()], outs=[ccg_out[:].opt()])
        nc.gpsimd.dma_start(
            sq_col[:], ccr_sq_out[0, :].rearrange("(j p) -> p j", p=P))

        # ---- Ev chunks (stat already resident; no input DMA here)
        for ch in range(NCH):
            cs = slice(ch * 512, (ch + 1) * 512)
            ev_ps = ppk.tile([P, 512], F32, tag="ev")
            for i2 in range(NT_I // 2):
                nc.tensor.matmul(ev_ps[:], wev_sb[:, 2 * i2:2 * i2 + 2, :],
                                 sts[ch][:, 2 * i2:2 * i2 + 2, :],
                                 start=(i2 == 0), stop=(i2 == NT_I // 2 - 1),
                                 perf_mode=DR)
            evbf = pw.tile([P, 512], BF16, tag="evbf")
            nc.scalar.activation(evbf[:], ev_ps[:], AF.Identity,
                                 bias=bevc_sb[:], scale=1.0 / WS)
            rw_ps = prow.tile([1, 512], F32, tag="row")
            nc.tensor.matmul(rw_ps[:], wwc_sb[:], evbf[:])
            rw_row = rowp.tile([1, 512], F32, tag="rwr")
            nc.vector.tensor_copy(rw_row[:], rw_ps[:])
            nc.gpsimd.dma_start(ccr_rw_in[0:1, cs], rw_row[:])
            tp_ps = ptp.tile([P, 512], BF16, tag="tp")
            for q in range(4):
                nc.tensor.transpose(tp_ps[:, q * P:(q + 1) * P],
                                    evbf[:, q * P:(q + 1) * P], id8_sb[:])
            evt8 = pw.tile([P, 512], F8, tag="evt8")
            nc.scalar.copy(evt8[:], tp_ps[:])
            nc.gpsimd.dma_start(ccv_in[:, cs], evt8[:])

        # ---- eksl scatter loads: 4 on the sync ring (its engine can
        # afford to block on the AG-ek wait), 4 on gpsimd after the
        # round-2 triggers.  NEVER on scalar: the trigger's collective
        # wait would stall every KV activation behind it.
        for m in range(4):
            nc.sync.dma_start(eksl[:, m, :], ccg_out[m])

        # ---- collectives round 2 (row AllReduce + Ev AllGather)
        nc.gpsimd.collective_compute(
            "AllReduce", mybir.AluOpType.add, replica_groups=GROUPS,
            ins=[ccr_rw_in[:].opt()], outs=[ccr_rw_out[:].opt()])
        nc.gpsimd.dma_start(
            wv_col[:], ccr_rw_out[0, :].rearrange("(j p) -> p j", p=P))
        nc.gpsimd.collective_compute(
            "AllGather", mybir.AluOpType.bypass, replica_groups=GROUPS,
            ins=[ccv_in[:].opt()], outs=[ccv_out[:].opt()])
        for m in range(4, NT_C):
            nc.gpsimd.dma_start(eksl[:, m, :], ccg_out[m])

    # ================= Phase KV ======================================
    with tc.tile_pool(name="pkv", bufs=3, space="PSUM") as pkv, \
         tc.tile_pool(name="pssk", bufs=1, space="PSUM") as pssk, \
         tc.tile_pool(name="pbc", bufs=1, space="PSUM") as pbc, \
         tc.tile_pool(name="sqp", bufs=2) as sqp, \
         tc.tile_pool(name="kvw", bufs=2) as kvw:
        # ---- k half (fp8 DoubleRow) + sum-of-squares
        ssk = pssk.tile([1, BL], F32)
        for m in range(NT_C):
            kv_ps = pkv.tile([P, BL], F32, tag="kv", name=f"k{m}")
            for i2 in range(NT_I // 2):
                nc.tensor.matmul(
                    kv_ps[:], wk8_sb[:, 2 * i2:2 * i2 + 2, m * P:(m + 1) * P],
                    x8_sb[:, 2 * i2:2 * i2 + 2, :],
                    start=(i2 == 0), stop=(i2 == NT_I // 2 - 1),
                    perf_mode=DR)
            nc.scalar.activation(kT_all[:, m, :], kv_ps[:], AF.Identity,
                                 bias=bkv_sb[:, m:m + 1], scale=1.0 / WS)
            sqk = sqp.tile([P, BL], BF16, tag="sqk")
            nc.scalar.activation(sqk[:], kv_ps[:], AF.Square,
                                 bias=bkv_sb[:, m:m + 1], scale=1.0 / WS)
            nc.tensor.matmul(ssk[:], ones_col[:], sqk[:],
                             start=(m == 0), stop=(m == NT_C - 1))
        # ---- norms: both Sqrts together (table 3), then warm table 0
        nk = kvw.tile([1, BL], F32, tag="nk")
        nc.scalar.activation(nk[:], ssk[:], AF.Sqrt, bias=epsb[:])
        nc.scalar.activation(nrm_sb[:], sq_col[:], AF.Sqrt,
                             bias=epsb_p[:], scale=256.0)
        nc.scalar.activation(warmo[:], warm[:], AF.Exp)
        invk = kvw.tile([1, BL], BF16, tag="invk")
        with nc.allow_low_precision(reason="bf16 1/|k| feeds fp8 kn"):
            nc.vector.reciprocal(invk[:], nk[:])
        nc.vector.reciprocal(inv_col[:], nrm_sb[:])
        # ---- v half (bf16), bc broadcast inserted after 2 tiles
        for m in range(NT_C):
            kv_ps = pkv.tile([P, BL], F32, tag="kv", name=f"v{m}")
            for i in range(NT_I):
                nc.tensor.matmul(
                    kv_ps[:], wvb_sb[:, i, m * P:(m + 1) * P],
                    xb_sb[:, i, :],
                    start=(i == 0), stop=(i == NT_I - 1))
            nc.scalar.activation(vr_all[:, m, :], kv_ps[:], AF.Relu,
                                 bias=bkv_sb[:, NT_C + m:NT_C + m + 1])
            if m == 1:
                bc = pbc.tile([P, BL], F32)
                nc.tensor.matmul(bc[:], s16_row[:], invk[:])
                for mm in range(NT_C):
                    nc.vector.tensor_mul(kn_all[:, mm, :],
                                         kT_all[:, mm, :], bc[:])

    # lhs2 col 32 = wEv gate row (col 0 stays the memset 1.0)
    nc.vector.tensor_copy(lhs2[:, :, 32], wv_col[:])

    # ================= Fused SIM + GATE + WF =========================
    with tc.tile_pool(name="gw", bufs=3) as gw, \
         tc.tile_pool(name="esw", bufs=9) as esw, \
         tc.tile_pool(name="psim", bufs=3, space="PSUM") as psim, \
         tc.tile_pool(name="pg", bufs=2, space="PSUM") as pg, \
         tc.tile_pool(name="pbc2", bufs=2, space="PSUM") as pbc2:
        es_hist = {}
        sk_hist = {}

        def emit_wf(kk):
            # deferred one iteration: bcs broadcast + wf multiplies for
            # prototype kk (gate chain long since finished -> no PE stall)
            bcs = pbc2.tile([P, BL], F32, tag="bcs")
            nc.tensor.matmul(bcs[:], h64_row[:], sk_hist[kk][:])
            for j in range(TPK):
                nc.vector.tensor_mul(wf_all[:, kk * TPK + j, :],
                                     es_hist[(kk, j)][:], bcs[:])

        for k in range(K):
            ks0 = k * 512
            gg = pg.tile([33, BL], F32, tag="gg")
            for j in range(TPK):
                kt = k * TPK + j
                ps = psim.tile([P, BL], F32, tag="sim")
                for m2 in range(NT_C // 2):
                    nc.tensor.matmul(
                        ps[:],
                        eksl[:, 2 * m2:2 * m2 + 2,
                             ks0 + j * P:ks0 + (j + 1) * P],
                        kn_all[:, 2 * m2:2 * m2 + 2, :],
                        start=(m2 == 0), stop=(m2 == NT_C // 2 - 1),
                        perf_mode=DR)
                es = esw.tile([P, BL], BF16, tag="es", name=f"es{kt}")
                nc.scalar.activation(es[:], ps[:], AF.Exp,
                                     scale=inv_col[:, kt:kt + 1])
                es_hist[(k, j)] = es
                nc.tensor.matmul(gg[:], lhs2[:, kt, 0:33], es[:],
                                 start=(j == 0), stop=(j == TPK - 1))
            # gate chain: rs=1/sum, tg=gtg*rs, sigmoid via tanh,
            # sk = (1+t)*rs (*64 folded into the broadcast row)
            rs = gw.tile([1, BL], F32, tag="rs")
            nc.vector.reciprocal(rs[:], gg[0:1, :])
            tg = gw.tile([1, BL], F32, tag="tg")
            nc.vector.tensor_mul(tg[:], gg[32:33, :], rs[:])
            tt = gw.tile([1, BL], F32, tag="tt")
            nc.scalar.activation(tt[:], tg[:], AF.Tanh,
                                 scale=0.5, bias=bwh_sb[0:1, 0:1])
            sk2 = gw.tile([1, BL], BF16, tag="sk2")
            nc.vector.scalar_tensor_tensor(
                sk2[:], tt[:], 1.0, rs[:],
                op0=mybir.AluOpType.add, op1=mybir.AluOpType.mult)
            sk_hist[k] = sk2
            if k > 0:
                emit_wf(k - 1)
        emit_wf(K - 1)

    # ================= Phase FE + OUT ================================
    with tc.tile_pool(name="evp", bufs=3) as evp, \
         tc.tile_pool(name="ow", bufs=1) as ow, \
         tc.tile_pool(name="pfe", bufs=3, space="PSUM") as pfe, \
         tc.tile_pool(name="pout", bufs=1, space="PSUM") as pout:
        po = pout.tile([K, BL], F32)
        for j in range(NT_C):
            nc.tensor.matmul(po[:], wo_sb[:, j * K:(j + 1) * K],
                             vr_all[:, j, :],
                             start=(j == 0), stop=False)
        evtts = {}
        for mc in range(2):
            evtt = evp.tile([P, NT_KN, P], F8, tag="evt", name=f"ev{mc}")
            eng = nc.sync if mc % 2 == 0 else nc.gpsimd
            eng.dma_start(evtt[:], ccv_out[mc])
            evtts[mc] = evtt
        for mc in range(NT_C):
            evtt = evtts.pop(mc)
            if mc + 2 < NT_C:
                nxt = evp.tile([P, NT_KN, P], F8, tag="evt",
                               name=f"ev{mc + 2}")
                eng = nc.sync if mc % 2 == 0 else nc.gpsimd
                eng.dma_start(nxt[:], ccv_out[mc + 2])
                evtts[mc + 2] = nxt
            ps = pfe.tile([P, BL], F32, tag="feps")
            for t2 in range(NT_KN // 2):
                nc.tensor.matmul(
                    ps[:], evtt[:, 2 * t2:2 * t2 + 2, :],
                    wf_all[:, 2 * t2:2 * t2 + 2, :],
                    start=(t2 == 0), stop=(t2 == NT_KN // 2 - 1),
                    perf_mode=DR)
            nc.scalar.activation(fr_all[:, mc, :], ps[:], AF.Relu,
                                 scale=1.0 / WFS)
            nc.tensor.matmul(po[:], wo_sb[:, (NT_C + mc) * K:
                                          (NT_C + mc + 1) * K],
                             fr_all[:, mc, :],
                             start=False, stop=(mc == NT_C - 1))
        osb = ow.tile([K, BL], F32)
        nc.scalar.activation(osb[:], po[:], AF.Identity, bias=bout_sb[:])
        nc.sync.dma_start(outT[:], osb[:])

    for f in reversed(frees):
        f()

    tc_cm.__exit__(None, None, None)
    nc.compile()
    return nc


def _tile_rows(a):
    """[NT_I*P, n] -> [P, NT_I, n]: row (i*P + p) -> [p, i]."""
    n = a.shape[1]
    return np.ascontiguousarray(
        a.reshape(NT_I, P, n).transpose(1, 0, 2))


def _f8(a):
    return np.clip(a, -240.0, 240.0).astype(ml_dtypes.float8_e4m3)


def _host_prep(inputs):
    bf = ml_dtypes.bfloat16
    x_last = np.asarray(inputs["x"])[:, -1, :]  # [B, CH] f32
    wekT = np.asarray(inputs["WEk"]).T * WS  # [CH, C]
    wevT = np.asarray(inputs["WEv"]).T * WS
    wvT = np.asarray(inputs["Wv"]).T  # [CH, C]
    stat2 = np.asarray(inputs["static"]).transpose(1, 0, 2).reshape(CH, KN)
    shared = {
        "wk8t": _f8(_tile_rows(np.asarray(inputs["Wk"]).T * WS)),
        "wvbt": np.stack([
            _tile_rows(np.ascontiguousarray(wvT[:, m * P:(m + 1) * P]))
            for m in range(NT_C)]).astype(bf),
        "statt": _f8(np.stack([
            _tile_rows(np.ascontiguousarray(stat2[:, c * 512:(c + 1) * 512]))
            for c in range(NCH)])),
        "bkv": np.ascontiguousarray(
            np.concatenate([inputs["bk"], inputs["bv"]]).reshape(NT_KV, P).T),
        "id8": np.eye(P, dtype=bf),
        "wout": np.ascontiguousarray(
            np.asarray(inputs["Wout"]).T.reshape(NT_KV, P, K)
            .transpose(1, 0, 2).reshape(P, NT_KV * K)).astype(bf),
        "bwh": np.asarray(inputs["bw"], dtype=np.float32).reshape(1, 1) * 0.5,
        "boutt": np.asarray(inputs["bout"], dtype=np.float32).reshape(K, 1),
    }
    in_maps = []
    for r in range(NCORES):
        cslc = slice(r * P, (r + 1) * P)
        m = dict(shared)
        xs = np.ascontiguousarray(x_last[r * BL:(r + 1) * BL].T)
        m["xbt"] = _tile_rows(xs).astype(bf)
        m["wekt"] = _f8(_tile_rows(np.ascontiguousarray(wekT[:, cslc])))
        m["wevt"] = _f8(_tile_rows(np.ascontiguousarray(wevT[:, cslc])))
        m["bekc"] = np.ascontiguousarray(
            np.asarray(inputs["bEk"], dtype=np.float32)[cslc].reshape(P, 1))
        m["bevc"] = np.ascontiguousarray(
            np.asarray(inputs["bEv"], dtype=np.float32)[cslc].reshape(P, 1))
        m["wwc"] = np.ascontiguousarray(
            np.asarray(inputs["Ww"])[0, cslc].reshape(P, 1)).astype(bf)
        in_maps.append(m)
    return in_maps


def kernel(**inputs):
    if "nc" not in _CACHE:
        _CACHE["nc"] = _build_nc()
    nc = _CACHE["nc"]
    in_maps = _host_prep(inputs)
    res = bass_utils.run_bass_kernel_spmd(
        nc, in_maps, core_ids=list(range(NCORES)), trace=False)
    out = np.concatenate(
        [res.results[r]["outT"].T for r in range(NCORES)], axis=0)
    return np.ascontiguousarray(out[:, :, None], dtype=np.float32)
